# revision 1
# baseline (speedup 1.0000x reference)
"""Transformer decoder layer (causal self-attn + cross-attn + FFN, 3 post-LNs)
on 8 Trainium2 NeuronCores.

Sharding: 2-way data parallel (batch) x 4-way tensor parallel.
  core c: batch g = c // 4, TP rank r = c % 4.
  - attention: 4 of 16 heads per core (wq/wk/wv column slice 256, wo row
    slice 256), AllReduce[group of 4] after the output projection.
  - FFN: w1 column slice 1024, w2 row slice 1024, AllReduce after w2.
  - residual: each core folds 0.25*residual (+ bias/4) into its partial
    before the AllReduce, so the AllReduce output is directly the LN input.
  - LayerNorms computed redundantly on each core of the group.

On-chip layouts (per core, S tokens):
  feature-major "transposed" activations xT: [128, 8, S] bf16 (E on partitions)
  qT/kT: [128, 2, S] bf16 (head-dim on partitions, 4 heads x 64)
  v:     [128, TB, 4, 65] bf16 token-major, col 64 = ones (rowsum trick)
  attention scores sT: [128 k, 512 q] blocks, softmax along k via exp +
    ones-column rowsums; normalization folded into the o-eviction scale.

Matmul operands are bf16 (f32 PSUM accumulation); residual / LN / collective
payloads are f32.
"""

import numpy as np
import ml_dtypes

import concourse.bass as bass
import concourse.bacc as bacc
import concourse.tile as tile
from concourse import mybir
from concourse import bass_utils
from concourse.masks import make_identity

F32 = mybir.dt.float32
BF16 = mybir.dt.bfloat16
AF = mybir.ActivationFunctionType
ALU = mybir.AluOpType

E = 1024
H_PER_CORE = 4      # heads per core (16 / 4 TP ranks)
DK = 64
QKV = H_PER_CORE * DK   # 256
FFN_SLICE = 1024        # 4096 / 4 TP ranks
EB = E // 128           # 8 E partition-blocks
NEG_BIG = -30000.0      # additive mask value (exp -> 0 in f32)


def _ts(i, n):
    return slice(i * n, (i + 1) * n)


def _pbcast(ap, p=128):
    """Broadcast a 1D DRAM AP across p partitions (partition step 0)."""
    return bass.AP(tensor=ap.tensor, offset=ap.offset, ap=[[0, p]] + list(ap.ap))


def build_decoder_nc(S: int, num_devices: int = 8, stop_after: str | None = None):
    """Build the SPMD Bass program for one core (sequence length S)."""
    assert S % 512 == 0
    TB = S // 128          # token blocks
    QT = S // 512          # query tiles

    nc = bacc.Bacc("TRN2", target_bir_lowering=False, debug=False,
                   num_devices=num_devices)

    # ---------------- DRAM I/O ----------------
    din = {}

    def inp(name, shape, dt):
        din[name] = nc.dram_tensor(name, list(shape), dt, kind="ExternalInput")
        return din[name]

    x0_f = inp("x0_f", [S, E], F32)          # input (this batch), f32
    x0_b = inp("x0_b", [S, E], BF16)         # same, bf16 (for DMA transpose)
    enc_b = inp("enc_b", [S, E], BF16)       # encoder output, bf16

    for p in ("sa", "ca"):
        inp(f"{p}_wq", [E, QKV], BF16)
        inp(f"{p}_wk", [E, QKV], BF16)
        inp(f"{p}_wv", [E, QKV], BF16)
        inp(f"{p}_wo", [QKV, E], BF16)
        inp(f"{p}_bq", [QKV], F32)
        inp(f"{p}_bk", [QKV], F32)
        inp(f"{p}_bv", [QKV], F32)
        inp(f"{p}_bo4", [E], F32)            # bo / group_size
    inp("w1", [E, FFN_SLICE], BF16)
    inp("b1", [FFN_SLICE], F32)
    inp("w2", [FFN_SLICE, E], BF16)
    inp("b24", [E], F32)                     # b2 / group_size
    for i in (1, 2, 3):
        inp(f"ln{i}_g", [E], F32)
        inp(f"ln{i}_b", [E], F32)
    inp("cmask", [4, 128, 512], BF16)        # causal straddle masks

    G = 4 if num_devices >= 8 else num_devices
    out = nc.dram_tensor("out", [S // G, E], F32, kind="ExternalOutput")

    rg = [[0, 1, 2, 3], [4, 5, 6, 7]][: max(1, num_devices // 4)]
    if num_devices < 8:
        rg = [list(range(num_devices))]

    with tile.TileContext(nc) as tc:
        _emit(tc, din, out, S, TB, QT, rg, stop_after)

    nc.compile()
    return nc


PHASES = ["xt", "saqkv", "saattn", "sa", "cakv", "ar1", "ln1",
          "ca", "ar2", "ln2", "ffn1", "ffn2", "full"]


def _emit(tc, din, out, S, TB, QT, rg, stop_after=None):
    nc = tc.nc

    def cut(phase):
        # True -> caller should emit the early-exit and stop
        return stop_after == phase

    with (
        tc.tile_pool(name="const", bufs=1) as const,
        tc.tile_pool(name="wpool", bufs=1) as wpool,
        tc.tile_pool(name="xt", bufs=1) as xt_pool,
        tc.tile_pool(name="qkv", bufs=1) as qkv_pool,
        tc.tile_pool(name="attn", bufs=2) as attn_pool,
        tc.tile_pool(name="opool", bufs=1) as o_pool,
        tc.tile_pool(name="lnp", bufs=2) as lnp,
        tc.tile_pool(name="stat", bufs=8) as stat,
        tc.tile_pool(name="pp", bufs=2, space="PSUM") as pp,
        tc.tile_pool(name="ps_s", bufs=2, space="PSUM") as ps_s,
        tc.tile_pool(name="ps_o", bufs=2, space="PSUM") as ps_o,
        tc.tile_pool(name="ps_t", bufs=2, space="PSUM") as ps_t,
        tc.tile_pool(name="dram", bufs=1, space="DRAM") as dram,
    ):
        # ---------------- constants ----------------
        ident = const.tile([128, 128], BF16)
        make_identity(nc, ident)
        eps_t = const.tile([128, 1], F32)
        nc.vector.memset(eps_t, 1e-12)
        cmask = const.tile([128, 4, 512], BF16)
        nc.sync.dma_start(out=cmask, in_=din["cmask"].ap().rearrange("i p q -> p i q"))

        _bcast_cache = {}

        def bcast(name, dt=F32, tag=""):
            if name in _bcast_cache:
                return _bcast_cache[name]
            t = const.tile([128, E], dt, name=f"bc_{name}", tag=tag)
            nc.sync.dma_start(out=t, in_=_pbcast(din[name].ap()))
            _bcast_cache[name] = t
            return t

        def ln_g(i):
            return bcast(f"ln{i}_g", tag="lng")

        def ln_b(i):
            return bcast(f"ln{i}_b", tag="lnb")

        def bo4(p):
            return bcast(f"{p}_bo4", tag="bo4")

        def b24_b():
            return bcast("b24", tag="bo4")

        # per-partition bias tiles
        def pp_bias(name, nj):
            t = const.tile([128, nj], F32, name=f"ppb_{name}")
            nc.sync.dma_start(out=t, in_=din[name].ap().rearrange("(j p) -> p j", p=128))
            return t

        bq = {p: pp_bias(f"{p}_bq", 2) for p in ("sa", "ca")}
        bk = {p: pp_bias(f"{p}_bk", 2) for p in ("sa", "ca")}
        b1_t = pp_bias("b1", 8)
        def bv_b(p):
            t = const.tile([128, QKV], F32, name=f"bvb_{p}", tag="bvb")
            nc.sync.dma_start(out=t, in_=_pbcast(din[f"{p}_bv"].ap()))
            return t

        # ---------------- DRAM scratch ----------------
        G = len(rg[0])
        ar_in, ar_out = {}, {}
        for i in (1, 2):
            ar_in[i] = dram.tile([S, E], BF16, name=f"ar{i}_in")
            ar_out[i] = dram.tile([S, E], BF16, name=f"ar{i}_out")
        ar_in[3] = dram.tile([S, E], F32, name="ar3_in")
        ar_out[3] = dram.tile([S // G, E], F32, name="rs3_out")
        x_res = {1: dram.tile([S, E], F32, name="x1_dram"),
                 2: dram.tile([S, E], F32, name="x2_dram")}
        x_bf = {1: dram.tile([S, E], BF16, name="x1bf_dram"),
                2: dram.tile([S, E], BF16, name="x2bf_dram")}

        # ---------------- helpers ----------------
        def load_w_qkv(pref):
            w = {}
            for nm in ("wq", "wk", "wv"):
                t = wpool.tile([128, EB, QKV], BF16, tag=nm, name=f"{pref}_{nm}_sb")
                nc.sync.dma_start(out=t, in_=din[f"{pref}_{nm}"].ap().rearrange(
                    "(eb p) m -> p eb m", p=128))
                w[nm] = t
            return w

        def load_w_o(pref):
            t = wpool.tile([128, 2, E], BF16, tag="wo", name=f"{pref}_wo_sb")
            nc.sync.dma_start(out=t, in_=din[f"{pref}_wo"].ap().rearrange(
                "(j p) n -> p j n", p=128))
            return t

        def dma_transpose_in(dst, src_dram):
            # src [S, E] (2-byte) -> dst [128, EB, S] feature-major
            for eb in range(EB):
                nc.sync.dma_start_transpose(dst[:, eb, :], src_dram[:, _ts(eb, 128)])

        def proj_qk(xT, w, b, dst):
            # dst [128, 2, S] bf16 = (w.T @ x.T) + b   (feature-major)
            for j in range(2):
                for tt in range(QT):
                    ps = pp.tile([128, 512], F32, tag="pp")
                    for eb in range(EB):
                        nc.tensor.matmul(ps, w[:, eb, _ts(j, 128)],
                                         xT[:, eb, _ts(tt, 512)],
                                         start=(eb == 0), stop=(eb == EB - 1))
                    nc.vector.tensor_scalar_add(dst[:, j, _ts(tt, 512)], ps,
                                                b[:, j:j + 1])

        def proj_v(xT, w, bvb, dst):
            # dst [128, TB, 4, 65] token-major v (+ ones column)
            nc.vector.memset(dst[:, :, :, 64:65], 1.0)
            for tb in range(TB):
                ps = pp.tile([128, QKV], F32, tag="pp")
                for eb in range(EB):
                    nc.tensor.matmul(ps, xT[:, eb, _ts(tb, 128)], w[:, eb, :],
                                     start=(eb == 0), stop=(eb == EB - 1))
                nc.vector.tensor_add(dst[:, tb, :, 0:64],
                                     ps.rearrange("p (h d) -> p h d", d=64), bvb)

        def attention(qT, kT, v, o_sb, causal):
            for h in range(H_PER_CORE):
                hp = slice((h % 2) * 64, (h % 2) * 64 + 64)
                j = h // 2
                for qt in range(QT):
                    kb_max = min(TB, 4 * qt + 4) if causal else TB
                    at = attn_pool.tile([128, TB, 512], BF16, tag="attn")
                    for kb in range(kb_max):
                        ps = ps_s.tile([128, 512], F32, tag="ps_s")
                        nc.tensor.matmul(ps, kT[hp, j, _ts(kb, 128)],
                                         qT[hp, j, _ts(qt, 512)],
                                         start=True, stop=True)
                        nc.scalar.activation(at[:, kb, :], ps, AF.Exp, scale=0.125)
                        if causal and kb >= 4 * qt:
                            nc.vector.tensor_mul(at[:, kb, :], at[:, kb, :],
                                                 cmask[:, kb - 4 * qt, :])
                    for qs in range(4):
                        po = ps_o.tile([128, 65], F32, tag="ps_o")
                        for kb in range(kb_max):
                            nc.tensor.matmul(po, at[:, kb, _ts(qs, 128)],
                                             v[:, kb, h, :],
                                             start=(kb == 0), stop=(kb == kb_max - 1))
                        rcp = stat.tile([128, 1], F32, tag="rcp")
                        nc.vector.reciprocal(rcp, po[:, 64:65])
                        nc.vector.tensor_scalar_mul(o_sb[:, qt * 4 + qs, h, :],
                                                    po[:, 0:64], rcp)

        def o_transpose(o_sb, oT):
            for tb in range(TB):
                for j in range(2):
                    pt = ps_t.tile([128, 128], BF16, tag="ps_t")
                    nc.tensor.transpose(pt, o_sb[:, tb, 2 * j:2 * j + 2, :], ident)
                    nc.vector.tensor_copy(oT[:, j, _ts(tb, 128)], pt)

        def out_proj(oT, wo, bo4_b, ar_dst):
            # bf16 partial = oT.T @ wo + bo/G -> ar_dst (residual added post-AR)
            for tb in range(TB):
                y = lnp.tile([128, E], BF16, tag="res_out")
                for nh in range(2):
                    ps = pp.tile([128, 512], F32, tag="pp")
                    for j in range(2):
                        nc.tensor.matmul(ps, oT[:, j, _ts(tb, 128)],
                                         wo[:, j, _ts(nh, 512)],
                                         start=(j == 0), stop=(j == 1))
                    nc.vector.tensor_add(y[:, _ts(nh, 512)], ps,
                                         bo4_b[:, _ts(nh, 512)])
                nc.sync.dma_start(out=ar_dst[_ts(tb, 128), :], in_=y)

        def all_reduce(i):
            nc.gpsimd.collective_compute(
                "AllReduce", ALU.add, replica_groups=rg,
                ins=[ar_in[i].opt()], outs=[ar_out[i].opt()])

        def reduce_scatter(i):
            nc.gpsimd.collective_compute(
                "ReduceScatter", ALU.add, replica_groups=rg,
                ins=[ar_in[i].opt()], outs=[ar_out[i].opt()])

        def layer_norm(i, make_bf, to_out=None, residual_src=None, n_blocks=None):
            # LN over (ar_out[i] [+ residual]); writes x_res[i]/x_bf[i] or `out`
            for tb in range(n_blocks if n_blocks is not None else TB):
                ld = lnp.tile([128, E], F32, tag="ln_io")
                if residual_src is not None:
                    arb = lnp.tile([128, E], BF16, tag="ln_bf")
                    nc.sync.dma_start(out=arb, in_=ar_out[i][_ts(tb, 128), :])
                    nc.sync.dma_start(out=ld, in_=residual_src[_ts(tb, 128), :])
                    nc.vector.tensor_add(ld, ld, arb)
                else:
                    nc.sync.dma_start(out=ld, in_=ar_out[i][_ts(tb, 128), :])
                st = stat.tile([128, 2, 6], F32, tag="bnst")
                for sg in range(2):
                    nc.vector.bn_stats(st[:, sg, :], ld[:, _ts(sg, 512)])
                mv = stat.tile([128, 2], F32, tag="bnmv")
                nc.vector.bn_aggr(mv, st)
                sd = stat.tile([128, 1], F32, tag="sd")
                nc.scalar.activation(sd, mv[:, 1:2], AF.Sqrt, bias=eps_t)
                rstd = stat.tile([128, 1], F32, tag="rstd")
                nc.vector.reciprocal(rstd, sd)
                nc.vector.tensor_scalar(ld, ld, mv[:, 0:1], rstd,
                                        ALU.subtract, ALU.mult)
                nc.vector.tensor_mul(ld, ld, ln_g(i))
                nc.vector.tensor_add(ld, ld, ln_b(i))
                xf = ld
                if to_out is not None:
                    nc.sync.dma_start(out=to_out[_ts(tb, 128), :], in_=xf)
                else:
                    nc.sync.dma_start(out=x_res[i][_ts(tb, 128), :], in_=xf)
                    if make_bf:
                        xb = lnp.tile([128, E], BF16, tag="ln_bf")
                        nc.vector.tensor_copy(xb, xf)
                        nc.sync.dma_start(out=x_bf[i][_ts(tb, 128), :], in_=xb)

        # ================= self-attention =================
        def finish():
            nc.sync.dma_start(out=out.ap(), in_=din["x0_f"].ap()[:S // len(rg[0]), :])

        if cut("null"):
            finish()
            return

        x0T = xt_pool.tile([128, EB, S], BF16, tag="xT", name="x0T")
        dma_transpose_in(x0T, din["x0_b"].ap())

        sa_w = load_w_qkv("sa")
        sa_wo = load_w_o("sa")

        qT = qkv_pool.tile([128, 2, S], BF16, tag="qT", name="sa_qT")
        kT = qkv_pool.tile([128, 2, S], BF16, tag="kT", name="sa_kT")
        v = qkv_pool.tile([128, TB, 4, 65], BF16, tag="v", name="sa_v")
        proj_qk(x0T, sa_w["wq"], bq["sa"], qT)
        proj_qk(x0T, sa_w["wk"], bk["sa"], kT)
        proj_v(x0T, sa_w["wv"], bv_b("sa"), v)

        if cut("saqkv"):
            finish()
            return

        # encoder transpose-load takes over x0T's slot once SA projections drain
        encT = xt_pool.tile([128, EB, S], BF16, tag="xT", name="encT")
        dma_transpose_in(encT, din["enc_b"].ap())

        if cut("xt"):
            finish()
            return

        o_sb = o_pool.tile([128, TB, 4, 64], BF16, tag="o", name="sa_o")
        attention(qT, kT, v, o_sb, causal=True)
        oT = qkv_pool.tile([128, 2, S], BF16, tag="qT", name="sa_oT")
        o_transpose(o_sb, oT)

        if cut("saattn"):
            finish()
            return
        out_proj(oT, sa_wo, bo4("sa"), ar_in[1])

        if cut("sa"):
            finish()
            return

        # cross-attention K/V from encoder (independent of AR1 -> overlaps it)
        ca_w = load_w_qkv("ca")
        ca_kT = qkv_pool.tile([128, 2, S], BF16, tag="kT", name="ca_kT")
        ca_v = qkv_pool.tile([128, TB, 4, 65], BF16, tag="v", name="ca_v")
        proj_qk(encT, ca_w["wk"], bk["ca"], ca_kT)
        proj_v(encT, ca_w["wv"], bv_b("ca"), ca_v)

        if cut("cakv"):
            finish()
            return

        all_reduce(1)

        if cut("ar1"):
            finish()
            return
        layer_norm(1, make_bf=True, residual_src=din["x0_f"].ap())

        # ================= cross-attention =================
        x1T = xt_pool.tile([128, EB, S], BF16, tag="xT", name="x1T")
        dma_transpose_in(x1T, x_bf[1])

        if cut("ln1"):
            finish()
            return
        ca_wo = load_w_o("ca")
        ca_qT = qkv_pool.tile([128, 2, S], BF16, tag="qT", name="ca_qT")
        proj_qk(x1T, ca_w["wq"], bq["ca"], ca_qT)

        ca_o = o_pool.tile([128, TB, 4, 64], BF16, tag="o", name="ca_o")
        attention(ca_qT, ca_kT, ca_v, ca_o, causal=False)
        ca_oT = qkv_pool.tile([128, 2, S], BF16, tag="qT", name="ca_oT")
        o_transpose(ca_o, ca_oT)
        out_proj(ca_oT, ca_wo, bo4("ca"), ar_in[2])

        if cut("ca"):
            finish()
            return

        # FFN weights load early (overlaps AR2)
        w1_sb = wpool.tile([128, EB, FFN_SLICE], BF16, tag="wk")
        nc.sync.dma_start(out=w1_sb, in_=din["w1"].ap().rearrange(
            "(eb p) m -> p eb m", p=128))
        w2_sb = wpool.tile([128, 8, E], BF16, tag="wq")
        nc.sync.dma_start(out=w2_sb, in_=din["w2"].ap().rearrange(
            "(fb p) n -> p fb n", p=128))

        all_reduce(2)

        if cut("ar2"):
            finish()
            return
        layer_norm(2, make_bf=True, residual_src=x_res[1])

        # ================= FFN =================
        x2T = xt_pool.tile([128, EB, S], BF16, tag="xT", name="x2T")
        dma_transpose_in(x2T, x_bf[2])

        if cut("ln2"):
            finish()
            return
        hT = xt_pool.tile([128, 8, S], BF16, tag="hT", name="hT")
        for fb in range(8):
            for tt in range(QT):
                ps = pp.tile([128, 512], F32, tag="pp")
                for eb in range(EB):
                    nc.tensor.matmul(ps, w1_sb[:, eb, _ts(fb, 128)],
                                     x2T[:, eb, _ts(tt, 512)],
                                     start=(eb == 0), stop=(eb == EB - 1))
                nc.scalar.activation(hT[:, fb, _ts(tt, 512)], ps, AF.Relu,
                                     bias=b1_t[:, fb:fb + 1])

        if cut("ffn1"):
            finish()
            return
        for tb in range(TB):
            res = lnp.tile([128, E], F32, tag="ln_io")
            nc.sync.dma_start(out=res, in_=x_res[2][_ts(tb, 128), :])
            nc.vector.scalar_tensor_tensor(res, res, 1.0 / len(rg[0]),
                                           b24_b(), ALU.mult, ALU.add)
            base = res
            y = lnp.tile([128, E], F32, tag="res_out")
            for nh in range(2):
                ps = pp.tile([128, 512], F32, tag="pp")
                for fb in range(8):
                    nc.tensor.matmul(ps, hT[:, fb, _ts(tb, 128)],
                                     w2_sb[:, fb, _ts(nh, 512)],
                                     start=(fb == 0), stop=(fb == 7))
                nc.vector.tensor_add(y[:, _ts(nh, 512)], base[:, _ts(nh, 512)], ps)
            nc.sync.dma_start(out=ar_in[3][_ts(tb, 128), :], in_=y)

        if cut("ffn2"):
            finish()
            return

        reduce_scatter(3)
        layer_norm(3, make_bf=False, to_out=out.ap(), n_blocks=TB // G)


# ====================== host side ======================

def make_causal_masks():
    # mask_i[pk, pq] = 1.0 if pk <= pq - 128*i else 0  (straddle blocks)
    m = np.zeros((4, 128, 512), dtype=np.float32)
    pk = np.arange(128)[:, None]
    pq = np.arange(512)[None, :]
    for i in range(4):
        m[i] = (pk <= pq - 128 * i).astype(np.float32)
    return m.astype(ml_dtypes.bfloat16)


def shard_inputs(inputs, num_devices=8):
    """Full inputs (reference.setup_inputs keys) -> per-core in_maps."""
    bf = ml_dtypes.bfloat16
    f32 = np.float32
    G = 4 if num_devices >= 8 else num_devices
    cmask = make_causal_masks()
    in_maps = []
    inp = {k: np.asarray(v) for k, v in inputs.items()}
    for c in range(num_devices):
        g = c // G if num_devices >= 8 else 0
        r = c % G
        qs = slice(r * QKV, (r + 1) * QKV)
        fs = slice(r * FFN_SLICE, (r + 1) * FFN_SLICE)
        x0 = inp["input"][g].astype(f32)
        m = {
            "x0_f": x0,
            "x0_b": x0.astype(bf),
            "enc_b": inp["encoder_output"][g].astype(bf),
            "w1": inp["ffn_w1"][:, fs].astype(bf),
            "b1": inp["ffn_b1"][fs].astype(f32),
            "w2": inp["ffn_w2"][fs, :].astype(bf),
            "b24": (inp["ffn_b2"] / G).astype(f32),
            "cmask": cmask,
        }
        for p in ("sa", "ca"):
            m[f"{p}_wq"] = inp[f"{p}_wq"][:, qs].astype(bf)
            m[f"{p}_wk"] = inp[f"{p}_wk"][:, qs].astype(bf)
            m[f"{p}_wv"] = inp[f"{p}_wv"][:, qs].astype(bf)
            m[f"{p}_wo"] = inp[f"{p}_wo"][qs, :].astype(bf)
            m[f"{p}_bq"] = inp[f"{p}_bq"][qs].astype(f32)
            m[f"{p}_bk"] = inp[f"{p}_bk"][qs].astype(f32)
            m[f"{p}_bv"] = inp[f"{p}_bv"][qs].astype(f32)
            m[f"{p}_bo4"] = (inp[f"{p}_bo"] / G).astype(f32)
        for i in (1, 2, 3):
            m[f"ln{i}_g"] = inp[f"ln{i}_g"].astype(f32)
            m[f"ln{i}_b"] = inp[f"ln{i}_b"].astype(f32)
        in_maps.append(m)
    return in_maps


_NC_CACHE = {}


def _get_nc(S):
    if S not in _NC_CACHE:
        _NC_CACHE[S] = build_decoder_nc(S)
    return _NC_CACHE[S]


def kernel(**inputs):
    x = np.asarray(inputs["input"])
    B, S, _ = x.shape
    nc = _get_nc(S)
    in_maps = shard_inputs(inputs)
    res = bass_utils.run_bass_kernel_spmd(nc, in_maps, core_ids=list(range(8)))
    outb = [np.concatenate([res.results[g * 4 + r]["out"] for r in range(4)], axis=0)
            for g in range(B)]
    return np.stack(outb, axis=0).astype(np.float32)



# revision 12
# speedup vs baseline: 1.2220x; 1.2220x over previous
"""Transformer decoder layer (causal self-attn + cross-attn + FFN, 3 post-LNs)
on 8 Trainium2 NeuronCores — token-parallel version, zero collectives.

Sharding: 2-way data parallel (batch) x 4-way query-token striping.
  core c: batch g = c // 4, stripe r = c % 4 owns the 128-row blocks
  {r, r+4, r+8, r+12} of the sequence (512 query tokens).
  - K/V are computed redundantly on every core from the full input /
    encoder_output (which each core holds) for all 16 heads.
  - out-projections and the FFN are complete per token -> no reductions.
  - causality is data-driven (cmask4 per core), so the instruction
    stream is identical on all cores (true SPMD).

On-chip layouts (per core):
  x0T/encT  [128, 8, 2048]  bf16   feature-major full activations
  xrowT     [128, 8, 512]   bf16   feature-major own-token activations
  kT        [128, 8, 2048]  bf16   head-dim on partitions (2 heads x 64)
  qT        [128, 8, 512]   bf16
  v         [128, 16, 16, 65] fp8  token-major V (+ ones col = rowsum)
  at        [128, 16, 512]  fp8    exp(scores), k-major
  poT       [65, 512] PSUM  f32    v.T @ at (row 64 = softmax denom Z)
  oT        [128, 8, 512]   bf16   normalized attention out, feature-major

Softmax normalization: rcp(Z) row broadcast down 64 partitions via a
rank-1 matmul (ones[1,64].T @ rz[1,512]), then one DVE multiply.
"""

import numpy as np
import ml_dtypes

import concourse.bass as bass
import concourse.bacc as bacc
import concourse.tile as tile
from concourse import mybir
from concourse import bass_utils
from concourse.masks import make_identity

F32 = mybir.dt.float32
BF16 = mybir.dt.bfloat16
FP8 = mybir.dt.float8e4
AF = mybir.ActivationFunctionType
ALU = mybir.AluOpType

E = 1024
EB = 8           # E / 128
H = 16
HP = 8           # head pairs
DK = 64
S = 2048
TB = 16          # full-token 128-blocks
TBQ = 4          # own-token 128-blocks
SQ = 512         # own query tokens


def _ts(i, n):
    return slice(i * n, (i + 1) * n)


def _pbcast(ap, p=128):
    """Broadcast a 1D DRAM AP across p partitions (partition step 0)."""
    return bass.AP(tensor=ap.tensor, offset=ap.offset, ap=[[0, p]] + list(ap.ap))


PHASES = ["null", "x0t", "saqkv", "saattn", "ln1", "cakv", "caattn",
          "ln2", "ffn1", "ffn2", "full"]


def build_decoder_nc(S_arg: int = S, num_devices: int = 8,
                     stop_after: str | None = None):
    assert S_arg == S
    nc = bacc.Bacc("TRN2", target_bir_lowering=False, debug=False,
                   num_devices=num_devices)

    din = {}

    def inp(name, shape, dt):
        din[name] = nc.dram_tensor(name, list(shape), dt, kind="ExternalInput")
        return din[name]

    inp("x0_b", [S, E], BF16)            # full input (batch g), bf16
    inp("x0row_b", [SQ, E], BF16)        # own stripes, bf16
    inp("x0res", [SQ, E], F32)           # own stripes + sa_bo (residual)
    inp("enc_b", [S, E], BF16)
    for p in ("sa", "ca"):
        inp(f"{p}_wkv", [E, 2 * E], BF16)    # [wk | wv]
        inp(f"{p}_wq", [E, E], BF16)
        inp(f"{p}_wo", [E, E], BF16)
        inp(f"{p}_bq", [E], F32)
        inp(f"{p}_bk", [E], F32)
        inp(f"{p}_bv", [E], BF16)        # bcast-loaded
    inp("ca_bo", [E], BF16)
    inp("w1", [E, 4 * E], BF16)
    inp("b1", [4 * E], F32)
    inp("w2", [4 * E, E], BF16)
    inp("b2", [E], BF16)
    for i in (1, 2, 3):
        inp(f"ln{i}_g", [E], BF16)
        inp(f"ln{i}_b", [E], BF16)
    inp("cmask4", [4, 128, 128], FP8)    # per-core causal block masks

    out = nc.dram_tensor("out", [SQ, E], F32, kind="ExternalOutput")

    with tile.TileContext(nc) as tc:
        _emit(tc, din, out, stop_after)

    nc.compile()
    return nc


def _emit(tc, din, out, stop_after=None):
    nc = tc.nc

    def cut(phase):
        return stop_after == phase

    def finish():
        nc.sync.dma_start(out=out.ap(), in_=din["x0res"].ap())

    from contextlib import ExitStack
    with ExitStack() as _es:
        _es.enter_context(nc.allow_low_precision(
            reason="fp8 attn weights / bf16 softmax rcp; validated by rel-err"))
        def _pool(**kw):
            return _es.enter_context(tc.tile_pool(**kw))
        const = _pool(name="const", bufs=1)
        xt_pool = _pool(name="xt", bufs=1)          # x0T / encT
        wkv_pool = _pool(name="wkv", bufs=1)        # wkv / w1-half
        wq_pool = _pool(name="wq", bufs=1)          # wq / wo
        kt_pool = _pool(name="kt", bufs=1)          # kT / w2-half
        v_pool = _pool(name="vp", bufs=1)           # v / ffn y
        qt_pool = _pool(name="qt", bufs=1)
        attn_pool = _pool(name="att", bufs=2)       # at / hT
        o_pool = _pool(name="ot", bufs=1)
        xrt_pool = _pool(name="xrt", bufs=1)        # xrowT
        res_pool = _pool(name="res", bufs=1)        # x1/x2 row bf16
        lnp = _pool(name="lnp", bufs=2)
        lnr = _pool(name="lnr", bufs=1)
        zp = _pool(name="zp", bufs=1)
        stat = _pool(name="stat", bufs=4)
        pp = _pool(name="pp", bufs=2, space="PSUM")
        ps_s = _pool(name="ps_s", bufs=2, space="PSUM")
        po_pool = _pool(name="po", bufs=2, space="PSUM")
        aux = _pool(name="aux", bufs=2, space="PSUM")
        # ---------------- constants ----------------
        ident = const.tile([128, 128], BF16)
        make_identity(nc, ident)
        eps_t = const.tile([128, 1], F32)
        nc.vector.memset(eps_t, 1e-12)
        ones64 = const.tile([1, 64], BF16)
        nc.vector.memset(ones64, 1.0)
        cmask4 = const.tile([128, 4, 128], FP8)
        nc.sync.dma_start(out=cmask4,
                          in_=din["cmask4"].ap().rearrange("i p q -> p i q"))

        _bc = {}

        def bcast(name, tag=None):
            if name not in _bc:
                t = const.tile([128, E], BF16, name=f"bc_{name}",
                               tag=tag or f"bc_{name}")
                nc.sync.dma_start(out=t, in_=_pbcast(din[name].ap()))
                _bc[name] = t
            return _bc[name]

        def pp_bias(name, nj, dt=F32):
            t = const.tile([128, nj], dt, name=f"ppb_{name}")
            nc.sync.dma_start(out=t,
                              in_=din[name].ap().rearrange("(j p) -> p j", p=128))
            return t

        bq = {p: pp_bias(f"{p}_bq", 8) for p in ("sa", "ca")}
        bk = {p: pp_bias(f"{p}_bk", 8) for p in ("sa", "ca")}
        b1_t = pp_bias("b1", 32)

        # ---------------- shared helpers ----------------
        def load_wkv(pref):
            t = wkv_pool.tile([128, EB, 2 * E], BF16, tag="wkv",
                              name=f"{pref}_wkv")
            nc.sync.dma_start(out=t, in_=din[f"{pref}_wkv"].ap().rearrange(
                "(eb p) m -> p eb m", p=128))
            return t

        def load_sq(name, tag="wq"):
            t = wq_pool.tile([128, EB, E], BF16, tag=tag, name=f"{name}_sb")
            nc.sync.dma_start(out=t, in_=din[name].ap().rearrange(
                "(eb p) m -> p eb m", p=128))
            return t

        def kv_proj(xT, wkv, bkt, bvb, kT, v):
            # kT [128, hp, S] bf16 ; v [128, tb, h, 65] fp8 (ones col set later)
            for hp in range(HP):
                for tt in range(S // 512):
                    ps = pp.tile([128, 512], F32, tag="pp")
                    for eb in range(EB):
                        nc.tensor.matmul(ps, wkv[:, eb, _ts(hp, 128)],
                                         xT[:, eb, _ts(tt, 512)],
                                         start=(eb == 0), stop=(eb == EB - 1))
                    nc.vector.tensor_scalar_add(kT[:, hp, _ts(tt, 512)], ps,
                                                bkt[:, hp:hp + 1])
            for tb in range(TB):
                for half in range(2):
                    ps = pp.tile([128, 512], F32, tag="pp")
                    for eb in range(EB):
                        nc.tensor.matmul(ps, xT[:, eb, _ts(tb, 128)],
                                         wkv[:, eb, E + half * 512:
                                             E + half * 512 + 512],
                                         start=(eb == 0), stop=(eb == EB - 1))
                    nc.vector.tensor_add(
                        v[:, tb, _ts(half, 8), 0:64],
                        ps.rearrange("p (h d) -> p h d", d=64),
                        bvb[:, _ts(half, 512)].rearrange("p (h d) -> p h d", d=64))

        def q_proj(xrT, wq, bqt, qT):
            for hp in range(HP):
                ps = pp.tile([128, 512], F32, tag="pp")
                for eb in range(EB):
                    nc.tensor.matmul(ps, wq[:, eb, _ts(hp, 128)],
                                     xrT[:, eb, :],
                                     start=(eb == 0), stop=(eb == EB - 1))
                nc.vector.tensor_scalar_add(qT[:, hp, :], ps, bqt[:, hp:hp + 1])

        def attention(qT, kT, v, oT, causal):
            for h in range(H):
                hs = slice((h % 2) * 64, (h % 2) * 64 + 64)
                hp = h // 2
                at = attn_pool.tile([128, TB, 512], FP8, tag="at")
                for kb in range(TB):
                    c0 = 128 * (kb // 4) if causal else 0
                    ps = ps_s.tile([128, 512], F32, tag="ps_s")
                    nc.tensor.matmul(ps[:, c0:], kT[hs, hp, _ts(kb, 128)],
                                     qT[hs, hp, c0:], start=True, stop=True)
                    nc.scalar.activation(at[:, kb, c0:], ps[:, c0:],
                                         AF.Exp, scale=0.125)
                    if causal:
                        nc.vector.tensor_mul(at[:, kb, c0:c0 + 128],
                                             at[:, kb, c0:c0 + 128],
                                             cmask4[:, kb % 4, :])
                pot = po_pool.tile([128, 512], F32, tag="po")
                for kb in range(TB):
                    c0 = 128 * (kb // 4) if causal else 0
                    nc.tensor.matmul(pot[0:65, c0:], v[:, kb, h, :],
                                     at[:, kb, c0:],
                                     start=(kb == 0), stop=(kb == TB - 1),
                                     skip_group_check=True)
                rzb = zp.tile([1, 512], BF16, tag="zb")
                nc.vector.reciprocal(rzb, pot[64:65, :])
                pb = aux.tile([128, 512], F32, tag="aux")
                nc.tensor.matmul(pb[0:64, :], ones64, rzb,
                                 start=True, stop=True)
                pbs = zp.tile([64, 512], BF16, tag="pbs")
                nc.scalar.copy(pbs, pb[0:64, :])
                nc.vector.tensor_tensor(oT[hs, hp, :], pot[0:64, :],
                                        pbs, ALU.mult)

        def layer_norm_block(ld, i, out_bf, tb):
            # in-place LN of ld [128, E]; writes bf16 copy to out_bf[:, tb, :]
            st = stat.tile([128, 2, 6], F32, tag="bnst")
            for sg in range(2):
                nc.vector.bn_stats(st[:, sg, :], ld[:, _ts(sg, 512)])
            mv = stat.tile([128, 2], F32, tag="bnmv")
            nc.vector.bn_aggr(mv, st)
            sd = stat.tile([128, 1], F32, tag="sd")
            nc.scalar.activation(sd, mv[:, 1:2], AF.Sqrt, bias=eps_t)
            rstd = stat.tile([128, 1], F32, tag="rstd")
            nc.vector.reciprocal(rstd, sd)
            nc.vector.tensor_scalar(ld, ld, mv[:, 0:1], rstd,
                                    ALU.subtract, ALU.mult)
            nc.vector.tensor_mul(ld, ld, bcast(f"ln{i}_g"))
            if out_bf is not None:
                nc.vector.tensor_add(out_bf[:, tb, :], ld, bcast(f"ln{i}_b"))
            return ld

        def out_proj_ln(oT, wo, i, residual, out_bf):
            # y = oT.T @ wo (+ residual [+ca_bo]) -> LN_i -> out_bf bf16
            for tb in range(TBQ):
                ld = lnp.tile([128, E], F32, tag="ln_io")
                if i == 1:
                    res = lnr.tile([128, E], F32, tag="ln_res")
                    nc.sync.dma_start(out=res,
                                      in_=din["x0res"].ap()[_ts(tb, 128), :])
                for ns in range(2):
                    ps = pp.tile([128, 512], F32, tag="pp")
                    for jb in range(EB):
                        nc.tensor.matmul(ps, oT[:, jb, _ts(tb, 128)],
                                         wo[:, jb, _ts(ns, 512)],
                                         start=(jb == 0), stop=(jb == EB - 1))
                    if i == 1:
                        nc.vector.tensor_add(ld[:, _ts(ns, 512)], ps,
                                             res[:, _ts(ns, 512)])
                    else:
                        nc.vector.scalar_tensor_tensor(
                            ld[:, _ts(ns, 512)], ps, 1.0,
                            bcast("ca_bo", tag="bob")[:, _ts(ns, 512)], ALU.mult, ALU.add)
                        nc.vector.tensor_add(ld[:, _ts(ns, 512)],
                                             ld[:, _ts(ns, 512)],
                                             residual[:, tb, _ts(ns, 512)])
                layer_norm_block(ld, i, out_bf, tb)

        def row_transpose(src_bf, dst_T):
            # src [128, TBQ, E] bf16 token-major -> dst [128, EB, SQ]
            for tb in range(TBQ):
                for eb in range(EB):
                    pt = aux.tile([128, 512], BF16, tag="aux")
                    nc.tensor.transpose(pt[:, 0:128], src_bf[:, tb, _ts(eb, 128)],
                                        ident)
                    nc.scalar.copy(dst_T[:, eb, _ts(tb, 128)], pt[:, 0:128])

        # ================= start =================
        if cut("null"):
            finish()
            return

        x0T = xt_pool.tile([128, EB, S], BF16, tag="xT", name="x0T")
        for eb in range(EB):
            nc.sync.dma_start_transpose(x0T[:, eb, :],
                                        din["x0_b"].ap()[:, _ts(eb, 128)])
        x0rT = xrt_pool.tile([128, EB, SQ], BF16, tag="xrT", name="x0rT")
        for eb in range(EB):
            nc.sync.dma_start_transpose(x0rT[:, eb, :],
                                        din["x0row_b"].ap()[:, _ts(eb, 128)])

        if cut("x0t"):
            finish()
            return

        sa_wkv = load_wkv("sa")
        sa_wq = load_sq("sa_wq")

        kT = kt_pool.tile([128, HP, S], BF16, tag="kT", name="sa_kT")
        v = v_pool.tile([128, TB, H, 65], FP8, tag="v", name="sa_v")
        nc.vector.memset(v[:, :, :, 64:65], 1.0)
        qT = qt_pool.tile([128, HP, SQ], BF16, tag="qT", name="sa_qT")

        kv_proj(x0T, sa_wkv, bk["sa"], bcast("sa_bv", tag="bv"), kT, v)
        q_proj(x0rT, sa_wq, bq["sa"], qT)

        # loads that overlap SA attention (slots freed by the projections)
        sa_wo = load_sq("sa_wo")
        encT = xt_pool.tile([128, EB, S], BF16, tag="xT", name="encT")
        for eb in range(EB):
            nc.sync.dma_start_transpose(encT[:, eb, :],
                                        din["enc_b"].ap()[:, _ts(eb, 128)])
        ca_wkv = load_wkv("ca")

        if cut("saqkv"):
            finish()
            return

        oT = o_pool.tile([128, HP, SQ], FP8, tag="oT", name="sa_oT")
        attention(qT, kT, v, oT, causal=True)

        if cut("saattn"):
            finish()
            return

        x1row_bf = res_pool.tile([128, TBQ, E], BF16, tag="res", name="xrow_bf")
        out_proj_ln(oT, sa_wo, 1, None, x1row_bf)

        if cut("ln1"):
            finish()
            return

        # ================= cross-attention =================
        x1rT = xrt_pool.tile([128, EB, SQ], BF16, tag="xrT", name="x1rT")
        row_transpose(x1row_bf, x1rT)

        ca_wq = load_sq("ca_wq")
        ca_kT = kt_pool.tile([128, HP, S], BF16, tag="kT", name="ca_kT")
        ca_v = v_pool.tile([128, TB, H, 65], FP8, tag="v", name="ca_v")
        nc.vector.memset(ca_v[:, :, :, 64:65], 1.0)
        ca_qT = qt_pool.tile([128, HP, SQ], BF16, tag="qT", name="ca_qT")

        kv_proj(encT, ca_wkv, bk["ca"], bcast("ca_bv", tag="bv"), ca_kT, ca_v)
        q_proj(x1rT, ca_wq, bq["ca"], ca_qT)

        ca_wo = load_sq("ca_wo")

        if cut("cakv"):
            finish()
            return

        ca_oT = o_pool.tile([128, HP, SQ], FP8, tag="oT", name="ca_oT")
        attention(ca_qT, ca_kT, ca_v, ca_oT, causal=False)

        # FFN weight loads overlap CA out-proj/LN
        w1h = [None, None]
        w1ap = din["w1"].ap().rearrange("(eb p) m -> p eb m", p=128)

        if cut("caattn"):
            finish()
            return

        out_proj_ln(ca_oT, ca_wo, 2, x1row_bf, x1row_bf)  # x2row overwrites

        if cut("ln2"):
            finish()
            return

        # ================= FFN =================
        x2rT = xrt_pool.tile([128, EB, SQ], BF16, tag="xrT", name="x2rT")
        row_transpose(x1row_bf, x2rT)

        y = v_pool.tile([128, TBQ, E], F32, tag="v", name="ffn_y")
        for p in range(2):
            w1p = wkv_pool.tile([128, EB, 2 * E], BF16, tag="wkv",
                                name=f"w1_{p}")
            nc.sync.dma_start(out=w1p, in_=w1ap[:, :, _ts(p, 2 * E)])
            w2p = kt_pool.tile([128, 16, E], BF16, tag="kT", name=f"w2_{p}")
            nc.sync.dma_start(
                out=w2p,
                in_=din["w2"].ap()[_ts(p, 2 * E), :].rearrange(
                    "(fb q) n -> q fb n", q=128))
            hT = attn_pool.tile([128, 16, 512], FP8, tag="at", name=f"hT_{p}")
            for hb in range(16):
                ps = pp.tile([128, 512], F32, tag="pp")
                for eb in range(EB):
                    nc.tensor.matmul(ps, w1p[:, eb, _ts(hb, 128)],
                                     x2rT[:, eb, :],
                                     start=(eb == 0), stop=(eb == EB - 1))
                gfb = p * 16 + hb
                nc.scalar.activation(hT[:, hb, :], ps, AF.Relu,
                                     bias=b1_t[:, gfb:gfb + 1])
            if cut("ffn1") and p == 0:
                finish()
                return
            for tb in range(TBQ):
                if p == 0:
                    for ns in range(2):
                        ps = pp.tile([128, 512], F32, tag="pp")
                        for fb in range(16):
                            nc.tensor.matmul(ps, hT[:, fb, _ts(tb, 128)],
                                             w2p[:, fb, _ts(ns, 512)],
                                             start=(fb == 0), stop=(fb == 15))
                        nc.vector.scalar_tensor_tensor(
                            y[:, tb, _ts(ns, 512)], ps, 1.0,
                            bcast("b2", tag="bob")[:, _ts(ns, 512)], ALU.mult, ALU.add)
                else:
                    ld = lnp.tile([128, E], F32, tag="ln_io")
                    for ns in range(2):
                        ps = pp.tile([128, 512], F32, tag="pp")
                        for fb in range(16):
                            nc.tensor.matmul(ps, hT[:, fb, _ts(tb, 128)],
                                             w2p[:, fb, _ts(ns, 512)],
                                             start=(fb == 0), stop=(fb == 15))
                        nc.vector.tensor_tensor(ld[:, _ts(ns, 512)],
                                                y[:, tb, _ts(ns, 512)], ps,
                                                ALU.add)
                    nc.vector.tensor_add(ld, ld, x1row_bf[:, tb, :])
                    layer_norm_block(ld, 3, None, tb)
                    nc.vector.tensor_add(ld, ld, bcast("ln3_b"))
                    nc.sync.dma_start(out=out.ap()[_ts(tb, 128), :], in_=ld)

        if cut("ffn2"):
            return


# ====================== host side ======================

def stripe_idx(r):
    blocks = [r, r + 4, r + 8, r + 12]
    return np.concatenate([np.arange(128 * b, 128 * b + 128) for b in blocks])


def make_cmask4(r):
    # i < r: pass; i == r: lower-tri straddle; i > r: blocked
    m = np.zeros((4, 128, 128), dtype=np.float32)
    pk = np.arange(128)[:, None]
    pq = np.arange(128)[None, :]
    for i in range(4):
        if i < r:
            m[i] = 1.0
        elif i == r:
            m[i] = (pk <= pq).astype(np.float32)
    return m.astype(ml_dtypes.float8_e4m3fn)


def shard_inputs(inputs, num_devices=8):
    bf = ml_dtypes.bfloat16
    f32 = np.float32
    inp = {k: np.asarray(v) for k, v in inputs.items()}
    in_maps = []
    for c in range(num_devices):
        g, r = c // 4, c % 4
        idx = stripe_idx(r)
        x0 = inp["input"][g].astype(f32)
        m = {
            "x0_b": x0.astype(bf),
            "x0row_b": x0[idx].astype(bf),
            "x0res": (x0[idx] + inp["sa_bo"][None, :]).astype(f32),
            "enc_b": inp["encoder_output"][g].astype(bf),
            "ca_bo": inp["ca_bo"].astype(bf),
            "w1": inp["ffn_w1"].astype(bf),
            "b1": inp["ffn_b1"].astype(f32),
            "w2": inp["ffn_w2"].astype(bf),
            "b2": inp["ffn_b2"].astype(bf),
            "cmask4": make_cmask4(r),
        }
        for p in ("sa", "ca"):
            m[f"{p}_wkv"] = np.concatenate(
                [inp[f"{p}_wk"], inp[f"{p}_wv"]], axis=1).astype(bf)
            m[f"{p}_wq"] = inp[f"{p}_wq"].astype(bf)
            m[f"{p}_wo"] = inp[f"{p}_wo"].astype(bf)
            m[f"{p}_bq"] = inp[f"{p}_bq"].astype(f32)
            m[f"{p}_bk"] = inp[f"{p}_bk"].astype(f32)
            m[f"{p}_bv"] = inp[f"{p}_bv"].astype(bf)
        for i in (1, 2, 3):
            m[f"ln{i}_g"] = inp[f"ln{i}_g"].astype(bf)
            m[f"ln{i}_b"] = inp[f"ln{i}_b"].astype(bf)
        in_maps.append(m)
    return in_maps


def unshard_outputs(per_core, B=2):
    """per_core: list/array of 8 x [SQ, E] -> [B, S, E]."""
    full = np.zeros((B, S, E), dtype=np.float32)
    for c in range(8):
        g, r = c // 4, c % 4
        full[g, stripe_idx(r)] = np.asarray(per_core[c], dtype=np.float32)
    return full


_NC_CACHE = {}


def _get_nc(S_arg):
    if S_arg not in _NC_CACHE:
        _NC_CACHE[S_arg] = build_decoder_nc(S_arg)
    return _NC_CACHE[S_arg]


def kernel(**inputs):
    x = np.asarray(inputs["input"])
    B, S_arg, _ = x.shape
    nc = _get_nc(S_arg)
    in_maps = shard_inputs(inputs)
    res = bass_utils.run_bass_kernel_spmd(nc, in_maps, core_ids=list(range(8)))
    return unshard_outputs([res.results[c]["out"] for c in range(8)], B=B)


# revision 14
# speedup vs baseline: 2.7922x; 2.2850x over previous
"""Transformer decoder layer (causal self-attn + cross-attn + FFN, 3 post-LNs)
on 8 Trainium2 NeuronCores — token-parallel version, zero collectives.

Sharding: 2-way data parallel (batch) x 4-way query-token striping.
  core c: batch g = c // 4, stripe r = c % 4 owns the 128-row blocks
  {r, r+4, r+8, r+12} of the sequence (512 query tokens).
  - K/V are computed redundantly on every core from the full input /
    encoder_output (which each core holds) for all 16 heads.
  - out-projections and the FFN are complete per token -> no reductions.
  - causality is data-driven (cmask4 per core), so the instruction
    stream is identical on all cores (true SPMD).

On-chip layouts (per core):
  x0T/encT  [128, 8, 2048]  bf16   feature-major full activations
  xrowT     [128, 8, 512]   bf16   feature-major own-token activations
  kT        [128, 8, 2048]  bf16   head-dim on partitions (2 heads x 64)
  qT        [128, 8, 512]   bf16
  v         [128, 16, 16, 65] fp8  token-major V (+ ones col = rowsum)
  at        [128, 16, 512]  fp8    exp(scores), k-major
  poT       [65, 512] PSUM  f32    v.T @ at (row 64 = softmax denom Z)
  oT        [128, 8, 512]   bf16   normalized attention out, feature-major

Softmax normalization: rcp(Z) row broadcast down 64 partitions via a
rank-1 matmul (ones[1,64].T @ rz[1,512]), then one DVE multiply.
"""

import numpy as np
import ml_dtypes

import concourse.bass as bass
import concourse.bacc as bacc
import concourse.tile as tile
from concourse import mybir
from concourse import bass_utils
from concourse.masks import make_identity

F32 = mybir.dt.float32
BF16 = mybir.dt.bfloat16
FP8 = mybir.dt.float8e4
AF = mybir.ActivationFunctionType
ALU = mybir.AluOpType

E = 1024
EB = 8           # E / 128
H = 16
HP = 8           # head pairs
DK = 64
S = 2048
TB = 16          # full-token 128-blocks
TBQ = 4          # own-token 128-blocks
SQ = 512         # own query tokens


def _ts(i, n):
    return slice(i * n, (i + 1) * n)


def _pbcast(ap, p=128):
    """Broadcast a 1D DRAM AP across p partitions (partition step 0)."""
    return bass.AP(tensor=ap.tensor, offset=ap.offset, ap=[[0, p]] + list(ap.ap))


PHASES = ["null", "x0t", "saqkv", "saattn", "ln1", "cakv", "caattn",
          "ln2", "ffn1", "ffn2", "full"]


def build_decoder_nc(S_arg: int = S, num_devices: int = 8,
                     stop_after: str | None = None):
    assert S_arg == S
    nc = bacc.Bacc("TRN2", target_bir_lowering=False, debug=False,
                   num_devices=num_devices)

    din = {}

    def inp(name, shape, dt):
        din[name] = nc.dram_tensor(name, list(shape), dt, kind="ExternalInput")
        return din[name]

    inp("x0_b", [S, E], BF16)            # full input (batch g), bf16
    inp("x0row_b", [SQ, E], BF16)        # own stripes, bf16
    inp("x0res", [SQ, E], F32)           # own stripes + sa_bo (residual)
    inp("enc_b", [S, E], BF16)
    for p in ("sa", "ca"):
        inp(f"{p}_wkv", [E, 2 * E], BF16)    # [wk | wv]
        inp(f"{p}_wq", [E, E], BF16)
        inp(f"{p}_wo", [E, E], BF16)
        inp(f"{p}_bq", [E], F32)
        inp(f"{p}_bk", [E], F32)
        inp(f"{p}_bv", [E], BF16)        # bcast-loaded
    inp("ca_bo", [E], BF16)
    inp("w1", [E, 4 * E], BF16)
    inp("b1", [4 * E], F32)
    inp("w2", [4 * E, E], BF16)
    inp("b2", [E], BF16)
    for i in (1, 2, 3):
        inp(f"ln{i}_g", [E], BF16)
        inp(f"ln{i}_b", [E], BF16)
    inp("cmask4", [4, 128, 128], FP8)    # per-core causal block masks

    out = nc.dram_tensor("out", [SQ, E], F32, kind="ExternalOutput")

    with tile.TileContext(nc) as tc:
        _emit(tc, din, out, stop_after)

    nc.compile()
    return nc


def _emit(tc, din, out, stop_after=None):
    nc = tc.nc

    def cut(phase):
        return stop_after == phase

    def finish():
        nc.sync.dma_start(out=out.ap(), in_=din["x0res"].ap())

    from contextlib import ExitStack
    with ExitStack() as _es:
        _es.enter_context(nc.allow_low_precision(
            reason="fp8 attn weights / bf16 softmax rcp; validated by rel-err"))
        def _pool(**kw):
            return _es.enter_context(tc.tile_pool(**kw))
        const = _pool(name="const", bufs=1)
        xt_pool = _pool(name="xt", bufs=1)          # x0T / encT
        wkv_pool = _pool(name="wkv", bufs=1)        # wkv / w1-half
        wq_pool = _pool(name="wq", bufs=1)          # wq / wo
        kt_pool = _pool(name="kt", bufs=1)          # kT / w2-half
        v_pool = _pool(name="vp", bufs=1)           # v / ffn y
        qt_pool = _pool(name="qt", bufs=1)
        attn_pool = _pool(name="att", bufs=2)       # at / hT
        o_pool = _pool(name="ot", bufs=1)
        xrt_pool = _pool(name="xrt", bufs=1)        # xrowT
        res_pool = _pool(name="res", bufs=1)        # x1/x2 row bf16
        lnp = _pool(name="lnp", bufs=2)
        lnr = _pool(name="lnr", bufs=1)
        zp = _pool(name="zp", bufs=1)
        stat = _pool(name="stat", bufs=4)
        pp = _pool(name="pp", bufs=2, space="PSUM")
        ps_s = _pool(name="ps_s", bufs=2, space="PSUM")
        po_pool = _pool(name="po", bufs=2, space="PSUM")
        aux = _pool(name="aux", bufs=2, space="PSUM")
        # ---------------- constants ----------------
        ident = const.tile([128, 128], BF16)
        make_identity(nc, ident)
        eps_t = const.tile([128, 1], F32)
        nc.vector.memset(eps_t, 1e-12)
        ones64 = const.tile([1, 64], BF16)
        nc.vector.memset(ones64, 1.0)
        cmask4 = const.tile([128, 4, 128], FP8)
        nc.sync.dma_start(out=cmask4,
                          in_=din["cmask4"].ap().rearrange("i p q -> p i q"))

        _bc = {}

        def bcast(name, tag=None):
            if name not in _bc:
                t = const.tile([128, E], BF16, name=f"bc_{name}",
                               tag=tag or f"bc_{name}")
                nc.sync.dma_start(out=t, in_=_pbcast(din[name].ap()))
                _bc[name] = t
            return _bc[name]

        def pp_bias(name, nj, dt=F32):
            t = const.tile([128, nj], dt, name=f"ppb_{name}")
            nc.sync.dma_start(out=t,
                              in_=din[name].ap().rearrange("(j p) -> p j", p=128))
            return t

        bq = {p: pp_bias(f"{p}_bq", 8) for p in ("sa", "ca")}
        bk = {p: pp_bias(f"{p}_bk", 8) for p in ("sa", "ca")}
        b1_t = pp_bias("b1", 32)

        # ---------------- shared helpers ----------------
        def load_wkv(pref):
            t = wkv_pool.tile([128, EB, 2 * E], BF16, tag="wkv",
                              name=f"{pref}_wkv")
            nc.sync.dma_start(out=t, in_=din[f"{pref}_wkv"].ap().rearrange(
                "(eb p) m -> p eb m", p=128))
            return t

        def load_sq(name, tag="wq"):
            t = wq_pool.tile([128, EB, E], BF16, tag=tag, name=f"{name}_sb")
            nc.sync.dma_start(out=t, in_=din[name].ap().rearrange(
                "(eb p) m -> p eb m", p=128))
            return t

        def kv_proj(xT, wkv, bkt, bvb, kT, v):
            # kT [128, hp, S] bf16 ; v [128, tb, h, 65] fp8 (ones col set later)
            for hp in range(HP):
                for tt in range(S // 512):
                    ps = pp.tile([128, 512], F32, tag="pp")
                    for eb in range(EB):
                        nc.tensor.matmul(ps, wkv[:, eb, _ts(hp, 128)],
                                         xT[:, eb, _ts(tt, 512)],
                                         start=(eb == 0), stop=(eb == EB - 1))
                    nc.vector.tensor_scalar_add(kT[:, hp, _ts(tt, 512)], ps,
                                                bkt[:, hp:hp + 1])
            for tb in range(TB):
                for half in range(2):
                    ps = pp.tile([128, 512], F32, tag="pp")
                    for eb in range(EB):
                        nc.tensor.matmul(ps, xT[:, eb, _ts(tb, 128)],
                                         wkv[:, eb, E + half * 512:
                                             E + half * 512 + 512],
                                         start=(eb == 0), stop=(eb == EB - 1))
                    nc.vector.tensor_add(
                        v[:, tb, _ts(half, 8), 0:64],
                        ps.rearrange("p (h d) -> p h d", d=64),
                        bvb[:, _ts(half, 512)].rearrange("p (h d) -> p h d", d=64))

        def q_proj(xrT, wq, bqt, qT):
            for hp in range(HP):
                ps = pp.tile([128, 512], F32, tag="pp")
                for eb in range(EB):
                    nc.tensor.matmul(ps, wq[:, eb, _ts(hp, 128)],
                                     xrT[:, eb, :],
                                     start=(eb == 0), stop=(eb == EB - 1))
                nc.vector.tensor_scalar_add(qT[:, hp, :], ps, bqt[:, hp:hp + 1])

        def attention(qT, kT, v, oT, causal):
            for h in range(H):
                hs = slice((h % 2) * 64, (h % 2) * 64 + 64)
                hp = h // 2
                at = attn_pool.tile([128, TB, 512], FP8, tag="at")
                for kb in range(TB):
                    c0 = 128 * (kb // 4) if causal else 0
                    ps = ps_s.tile([128, 512], F32, tag="ps_s")
                    nc.tensor.matmul(ps[:, c0:], kT[hs, hp, _ts(kb, 128)],
                                     qT[hs, hp, c0:], start=True, stop=True)
                    nc.scalar.activation(at[:, kb, c0:], ps[:, c0:],
                                         AF.Exp, scale=0.125)
                    if causal:
                        nc.gpsimd.tensor_mul(at[:, kb, c0:c0 + 128],
                                             at[:, kb, c0:c0 + 128],
                                             cmask4[:, kb % 4, :])
                pot = po_pool.tile([128, 512], F32, tag="po")
                for kb in range(TB):
                    c0 = 128 * (kb // 4) if causal else 0
                    nc.tensor.matmul(pot[0:65, c0:], v[:, kb, h, :],
                                     at[:, kb, c0:],
                                     start=(kb == 0), stop=(kb == TB - 1),
                                     skip_group_check=True)
                rzb = zp.tile([1, 512], BF16, tag="zb")
                nc.vector.reciprocal(rzb, pot[64:65, :])
                pb = aux.tile([128, 512], F32, tag="aux")
                nc.tensor.matmul(pb[0:64, :], ones64, rzb,
                                 start=True, stop=True)
                pbs = zp.tile([64, 512], BF16, tag="pbs")
                nc.scalar.copy(pbs, pb[0:64, :])
                nc.vector.tensor_tensor(oT[hs, hp, :], pot[0:64, :],
                                        pbs, ALU.mult)

        def layer_norm_block(ld, i, out_bf, tb):
            # in-place LN of ld [128, E]; writes bf16 copy to out_bf[:, tb, :]
            st = stat.tile([128, 2, 6], F32, tag="bnst")
            for sg in range(2):
                nc.vector.bn_stats(st[:, sg, :], ld[:, _ts(sg, 512)])
            mv = stat.tile([128, 2], F32, tag="bnmv")
            nc.vector.bn_aggr(mv, st)
            sd = stat.tile([128, 1], F32, tag="sd")
            nc.scalar.activation(sd, mv[:, 1:2], AF.Sqrt, bias=eps_t)
            rstd = stat.tile([128, 1], F32, tag="rstd")
            nc.vector.reciprocal(rstd, sd)
            nc.vector.tensor_scalar(ld, ld, mv[:, 0:1], rstd,
                                    ALU.subtract, ALU.mult)
            nc.vector.tensor_mul(ld, ld, bcast(f"ln{i}_g"))
            if out_bf is not None:
                nc.vector.tensor_add(out_bf[:, tb, :], ld, bcast(f"ln{i}_b"))
            return ld

        def out_proj_ln(oT, wo, i, residual, out_bf):
            # y = oT.T @ wo (+ residual [+ca_bo]) -> LN_i -> out_bf bf16
            for tb in range(TBQ):
                ld = lnp.tile([128, E], F32, tag="ln_io")
                if i == 1:
                    res = lnr.tile([128, E], F32, tag="ln_res")
                    nc.gpsimd.dma_start(out=res,
                                        in_=din["x0res"].ap()[_ts(tb, 128), :])
                for ns in range(2):
                    ps = pp.tile([128, 512], F32, tag="pp")
                    for jb in range(EB):
                        nc.tensor.matmul(ps, oT[:, jb, _ts(tb, 128)],
                                         wo[:, jb, _ts(ns, 512)],
                                         start=(jb == 0), stop=(jb == EB - 1))
                    if i == 1:
                        nc.vector.tensor_add(ld[:, _ts(ns, 512)], ps,
                                             res[:, _ts(ns, 512)])
                    else:
                        nc.vector.scalar_tensor_tensor(
                            ld[:, _ts(ns, 512)], ps, 1.0,
                            bcast("ca_bo", tag="bob")[:, _ts(ns, 512)], ALU.mult, ALU.add)
                        nc.vector.tensor_add(ld[:, _ts(ns, 512)],
                                             ld[:, _ts(ns, 512)],
                                             residual[:, tb, _ts(ns, 512)])
                layer_norm_block(ld, i, out_bf, tb)

        def row_transpose(src_bf, dst_T):
            # src [128, TBQ, E] bf16 token-major -> dst [128, EB, SQ]
            for tb in range(TBQ):
                for eb in range(EB):
                    pt = aux.tile([128, 512], BF16, tag="aux")
                    nc.tensor.transpose(pt[:, 0:128], src_bf[:, tb, _ts(eb, 128)],
                                        ident)
                    nc.scalar.copy(dst_T[:, eb, _ts(tb, 128)], pt[:, 0:128])

        # ================= start =================
        if cut("null"):
            finish()
            return

        x0T = xt_pool.tile([128, EB, S], BF16, tag="xT", name="x0T")
        for eb in range(EB):
            nc.sync.dma_start_transpose(x0T[:, eb, :],
                                        din["x0_b"].ap()[:, _ts(eb, 128)])
        x0rT = xrt_pool.tile([128, EB, SQ], BF16, tag="xrT", name="x0rT")
        for eb in range(EB):
            nc.sync.dma_start_transpose(x0rT[:, eb, :],
                                        din["x0row_b"].ap()[:, _ts(eb, 128)])

        if cut("x0t"):
            finish()
            return

        sa_wkv = load_wkv("sa")
        sa_wq = load_sq("sa_wq")

        kT = kt_pool.tile([128, HP, S], BF16, tag="kT", name="sa_kT")
        v = v_pool.tile([128, TB, H, 65], FP8, tag="v", name="sa_v")
        nc.vector.memset(v[:, :, :, 64:65], 1.0)
        qT = qt_pool.tile([128, HP, SQ], BF16, tag="qT", name="sa_qT")

        kv_proj(x0T, sa_wkv, bk["sa"], bcast("sa_bv", tag="bv"), kT, v)
        q_proj(x0rT, sa_wq, bq["sa"], qT)

        # loads that overlap SA attention (slots freed by the projections)
        sa_wo = load_sq("sa_wo")
        encT = xt_pool.tile([128, EB, S], BF16, tag="xT", name="encT")
        for eb in range(EB):
            nc.sync.dma_start_transpose(encT[:, eb, :],
                                        din["enc_b"].ap()[:, _ts(eb, 128)])
        ca_wkv = load_wkv("ca")

        if cut("saqkv"):
            finish()
            return

        oT = o_pool.tile([128, HP, SQ], FP8, tag="oT", name="sa_oT")
        attention(qT, kT, v, oT, causal=True)

        if cut("saattn"):
            finish()
            return

        x1row_bf = res_pool.tile([128, TBQ, E], BF16, tag="res", name="xrow_bf")
        out_proj_ln(oT, sa_wo, 1, None, x1row_bf)

        if cut("ln1"):
            finish()
            return

        # ================= cross-attention =================
        x1rT = xrt_pool.tile([128, EB, SQ], BF16, tag="xrT", name="x1rT")
        row_transpose(x1row_bf, x1rT)

        ca_wq = load_sq("ca_wq")
        ca_kT = kt_pool.tile([128, HP, S], BF16, tag="kT", name="ca_kT")
        ca_v = v_pool.tile([128, TB, H, 65], FP8, tag="v", name="ca_v")
        nc.vector.memset(ca_v[:, :, :, 64:65], 1.0)
        ca_qT = qt_pool.tile([128, HP, SQ], BF16, tag="qT", name="ca_qT")

        kv_proj(encT, ca_wkv, bk["ca"], bcast("ca_bv", tag="bv"), ca_kT, ca_v)
        q_proj(x1rT, ca_wq, bq["ca"], ca_qT)

        ca_wo = load_sq("ca_wo")

        if cut("cakv"):
            finish()
            return

        ca_oT = o_pool.tile([128, HP, SQ], FP8, tag="oT", name="ca_oT")
        attention(ca_qT, ca_kT, ca_v, ca_oT, causal=False)

        # FFN weight loads overlap CA out-proj/LN
        w1h = [None, None]
        w1ap = din["w1"].ap().rearrange("(eb p) m -> p eb m", p=128)

        if cut("caattn"):
            finish()
            return

        out_proj_ln(ca_oT, ca_wo, 2, x1row_bf, x1row_bf)  # x2row overwrites

        if cut("ln2"):
            finish()
            return

        # ================= FFN =================
        x2rT = xrt_pool.tile([128, EB, SQ], BF16, tag="xrT", name="x2rT")
        row_transpose(x1row_bf, x2rT)

        y = v_pool.tile([128, TBQ, E], F32, tag="v", name="ffn_y")
        for p in range(2):
            w1p = wkv_pool.tile([128, EB, 2 * E], BF16, tag="wkv",
                                name=f"w1_{p}")
            nc.sync.dma_start(out=w1p, in_=w1ap[:, :, _ts(p, 2 * E)])
            w2p = kt_pool.tile([128, 16, E], BF16, tag="kT", name=f"w2_{p}")
            nc.sync.dma_start(
                out=w2p,
                in_=din["w2"].ap()[_ts(p, 2 * E), :].rearrange(
                    "(fb q) n -> q fb n", q=128))
            hT = attn_pool.tile([128, 16, 512], FP8, tag="at", name=f"hT_{p}")
            for hb in range(16):
                ps = pp.tile([128, 512], F32, tag="pp")
                for eb in range(EB):
                    nc.tensor.matmul(ps, w1p[:, eb, _ts(hb, 128)],
                                     x2rT[:, eb, :],
                                     start=(eb == 0), stop=(eb == EB - 1))
                gfb = p * 16 + hb
                nc.scalar.activation(hT[:, hb, :], ps, AF.Relu,
                                     bias=b1_t[:, gfb:gfb + 1])
            if cut("ffn1") and p == 0:
                finish()
                return
            for tb in range(TBQ):
                if p == 0:
                    for ns in range(2):
                        ps = pp.tile([128, 512], F32, tag="pp")
                        for fb in range(16):
                            nc.tensor.matmul(ps, hT[:, fb, _ts(tb, 128)],
                                             w2p[:, fb, _ts(ns, 512)],
                                             start=(fb == 0), stop=(fb == 15))
                        nc.vector.scalar_tensor_tensor(
                            y[:, tb, _ts(ns, 512)], ps, 1.0,
                            bcast("b2", tag="bob")[:, _ts(ns, 512)], ALU.mult, ALU.add)
                else:
                    ld = lnp.tile([128, E], F32, tag="ln_io")
                    for ns in range(2):
                        ps = pp.tile([128, 512], F32, tag="pp")
                        for fb in range(16):
                            nc.tensor.matmul(ps, hT[:, fb, _ts(tb, 128)],
                                             w2p[:, fb, _ts(ns, 512)],
                                             start=(fb == 0), stop=(fb == 15))
                        nc.vector.tensor_tensor(ld[:, _ts(ns, 512)],
                                                y[:, tb, _ts(ns, 512)], ps,
                                                ALU.add)
                    nc.vector.tensor_add(ld, ld, x1row_bf[:, tb, :])
                    layer_norm_block(ld, 3, None, tb)
                    nc.vector.tensor_add(ld, ld, bcast("ln3_b"))
                    nc.sync.dma_start(out=out.ap()[_ts(tb, 128), :], in_=ld)

        if cut("ffn2"):
            return


# ====================== host side ======================

def stripe_idx(r):
    blocks = [r, r + 4, r + 8, r + 12]
    return np.concatenate([np.arange(128 * b, 128 * b + 128) for b in blocks])


def make_cmask4(r):
    # i < r: pass; i == r: lower-tri straddle; i > r: blocked
    m = np.zeros((4, 128, 128), dtype=np.float32)
    pk = np.arange(128)[:, None]
    pq = np.arange(128)[None, :]
    for i in range(4):
        if i < r:
            m[i] = 1.0
        elif i == r:
            m[i] = (pk <= pq).astype(np.float32)
    return m.astype(ml_dtypes.float8_e4m3fn)


def shard_inputs(inputs, num_devices=8):
    bf = ml_dtypes.bfloat16
    f32 = np.float32
    inp = {k: np.asarray(v) for k, v in inputs.items()}
    in_maps = []
    for c in range(num_devices):
        g, r = c // 4, c % 4
        idx = stripe_idx(r)
        x0 = inp["input"][g].astype(f32)
        m = {
            "x0_b": x0.astype(bf),
            "x0row_b": x0[idx].astype(bf),
            "x0res": (x0[idx] + inp["sa_bo"][None, :]).astype(f32),
            "enc_b": inp["encoder_output"][g].astype(bf),
            "ca_bo": inp["ca_bo"].astype(bf),
            "w1": inp["ffn_w1"].astype(bf),
            "b1": inp["ffn_b1"].astype(f32),
            "w2": inp["ffn_w2"].astype(bf),
            "b2": inp["ffn_b2"].astype(bf),
            "cmask4": make_cmask4(r),
        }
        for p in ("sa", "ca"):
            m[f"{p}_wkv"] = np.concatenate(
                [inp[f"{p}_wk"], inp[f"{p}_wv"]], axis=1).astype(bf)
            m[f"{p}_wq"] = inp[f"{p}_wq"].astype(bf)
            m[f"{p}_wo"] = inp[f"{p}_wo"].astype(bf)
            m[f"{p}_bq"] = inp[f"{p}_bq"].astype(f32)
            m[f"{p}_bk"] = inp[f"{p}_bk"].astype(f32)
            m[f"{p}_bv"] = inp[f"{p}_bv"].astype(bf)
        for i in (1, 2, 3):
            m[f"ln{i}_g"] = inp[f"ln{i}_g"].astype(bf)
            m[f"ln{i}_b"] = inp[f"ln{i}_b"].astype(bf)
        in_maps.append(m)
    return in_maps


def unshard_outputs(per_core, B=2):
    """per_core: list/array of 8 x [SQ, E] -> [B, S, E]."""
    full = np.zeros((B, S, E), dtype=np.float32)
    for c in range(8):
        g, r = c // 4, c % 4
        full[g, stripe_idx(r)] = np.asarray(per_core[c], dtype=np.float32)
    return full


_NC_CACHE = {}


def _get_nc(S_arg):
    if S_arg not in _NC_CACHE:
        _NC_CACHE[S_arg] = build_decoder_nc(S_arg)
    return _NC_CACHE[S_arg]


def kernel(**inputs):
    x = np.asarray(inputs["input"])
    B, S_arg, _ = x.shape
    nc = _get_nc(S_arg)
    in_maps = shard_inputs(inputs)
    res = bass_utils.run_bass_kernel_spmd(nc, in_maps, core_ids=list(range(8)))
    return unshard_outputs([res.results[c]["out"] for c in range(8)], B=B)
